# revision 25
# baseline (speedup 1.0000x reference)
"""DeepSet Jastrow factor kernel for Trainium2 (8 NeuronCores, data parallel).

Self-contained: builds a Bass/Tile kernel once (cached), shards the batch
across 8 cores, runs via run_bass_kernel_spmd, gathers the output.
"""

import sys
import types
from contextlib import ExitStack

sys.path.insert(0, "/opt/trn_rl_repo")

import numpy as np

import concourse.bass as bass
import concourse.tile as tile
from concourse import bacc, mybir
from concourse.bass_utils import run_bass_kernel_spmd

F32 = mybir.dt.float32
F32R = mybir.dt.float32r
BF16 = mybir.dt.bfloat16
AF = mybir.ActivationFunctionType
OP = mybir.AluOpType
AX = mybir.AxisListType

# ---------------- problem constants (hardcoded per spec) ----------------
B, N, D = 4096, 16, 3
P = N * (N - 1) // 2  # 120
NCORES = 8
BC = B // NCORES  # 512 samples per core
W = 128           # batch block width
NBLK = BC // W    # 4 blocks per core
CH = 24           # pairs per MLP chunk
NCHUNK = P // CH  # 5
FDC = CH * W      # 3072 tokens per chunk

F32EPS = float(np.finfo(np.float32).eps)
EPS2 = np.float64(0.04)        # EPS_FEAT^2, EPS_FEAT = 0.2
G2 = np.float64(0.09)          # GATE_R^2, GATE_R = 0.3

A_S1 = float(1.0 / EPS2)                      # 25
C_S1 = float((F32EPS + 2 * EPS2) / EPS2)      # ln(1+rt2) arg
C_W = float((F32EPS + EPS2) / EPS2)           # ln(rt2) arg
A_G = float(1.0 / G2)
C_G = float((F32EPS + G2) / G2)

IDX_I, IDX_J = np.triu_indices(N, k=1)
_spin = np.concatenate([np.zeros(N // 2), np.ones(N - N // 2)])
SPIN_MATCH = (_spin[IDX_I] == _spin[IDX_J]).astype(np.float32)
GAMMA = SPIN_MATCH * (1.0 / (D + 1)) + (1.0 - SPIN_MATCH) * (1.0 / (D - 1))
LNGAMMA = np.log(GAMMA).astype(np.float32)
SPIN_F = _spin.astype(np.float32)


def _gsel_const():
    """[48, 720] six stacked selection matrices: diff d0..2, cent d0..2."""
    G = np.zeros((6, 48, P), np.float32)
    for p in range(P):
        i, j = int(IDX_I[p]), int(IDX_J[p])
        for d in range(3):
            G[d, i * 3 + d, p] += 1.0
            G[d, j * 3 + d, p] -= 1.0
            G[3 + d, i * 3 + d, p] += 0.5
            G[3 + d, j * 3 + d, p] += 0.5
    return np.concatenate(list(G), axis=1)  # [48, 720]


# ---------------- device program ----------------

def _emit(tc, ctx, io):
    nc = tc.nc
    cst = ctx.enter_context(tc.tile_pool(name="cst", bufs=1))
    pm = ctx.enter_context(tc.tile_pool(name="pm", bufs=2))
    pm1 = ctx.enter_context(tc.tile_pool(name="pm1", bufs=1))
    pinp = ctx.enter_context(tc.tile_pool(name="pinp", bufs=2))
    hp = ctx.enter_context(tc.tile_pool(name="hp", bufs=2))
    drp = ctx.enter_context(tc.tile_pool(name="drp", bufs=2, space="DRAM"))
    mmps = ctx.enter_context(tc.tile_pool(name="mmps", bufs=2, space="PSUM"))
    smps = ctx.enter_context(tc.tile_pool(name="smps", bufs=2, space="PSUM"))

    def mm_ps(shape):
        return mmps.tile(shape, F32, tag="mm", name="mmtile")

    def sm_ps(shape):
        return smps.tile(shape, F32, tag="sm", name="smtile")

    # ---- load constants/weights to SBUF ----
    def load(name, shape, dtype=F32):
        t = cst.tile(shape, dtype, name=name)
        nc.sync.dma_start(out=t[:], in_=io[name][:])
        return t

    xTs = load("xT", [49, BC])
    gselS = load("gsel", [48, 720])
    w1f = load("w1", [13, 128])
    w2f = load("w2", [128, 128])
    w3f = load("w3", [128, 128])
    w4f = load("w4", [128, 128])
    w5f = load("w5p", [128, 64])
    b1S = load("b1", [128, 1])
    b2S = load("b2", [128, 1])
    b3S = load("b3", [128, 1])
    b4S = load("b4", [128, 1])
    wa1S = load("wa1bc", [120, 16])
    cbS = load("c1ba0", [120, 32])
    pmcS = load("pmc", [120, 2])
    gspS = load("gsp", [49, 1024])
    ws2S = load("ws2", [64, 64])
    bs2S = load("bs2", [64, 1])
    ws3S = load("ws3", [64, 16])
    apsS = load("aps", [32, 128])
    apaS = load("apa", [32, 128])
    aspS = load("asp", [16, 128])
    ascS = load("asc", [4, 128])
    br1S = load("br1", [128, 1])
    wr2S = load("wr2", [128, 128])
    br2S = load("br2", [128, 1])
    wr3S = load("wr3", [128, 128])
    br3S = load("br3", [128, 1])
    wr4S = load("wr4", [128, 1])
    br4S = load("br4", [1, 1])

    # fp32r-rounded pair-MLP weights
    def round_r(src, shape, name):
        t = cst.tile(shape, BF16, name=name)
        nc.vector.tensor_copy(out=t[:], in_=src[:])
        return t

    w1s = round_r(w1f, [13, 128], "w1s")
    w2s = round_r(w2f, [128, 128], "w2s")
    w3s = round_r(w3f, [128, 128], "w3s")
    w4s = round_r(w4f, [128, 128], "w4s")
    w5s = round_r(w5f, [128, 64], "w5s")

    ones120 = cst.tile([120, 1], F32)
    nc.vector.memset(ones120[:], 1.0)
    ones48 = cst.tile([48, 1], F32)
    nc.vector.memset(ones48[:], 1.0)
    onesr = cst.tile([120, 1], BF16)
    nc.vector.tensor_copy(out=onesr[:], in_=ones120[:])

    def cbias(val, name):
        t = cst.tile([120, 1], F32, name=name)
        nc.vector.memset(t[:], val)
        return t

    ones_row = cst.tile([1, 32], F32)
    nc.vector.memset(ones_row[:], 1.0)
    cb16 = cst.tile([120, 32], BF16)
    nc.vector.tensor_copy(out=cb16[:], in_=cbS[:])
    wa116 = cst.tile([120, 16], BF16)
    nc.vector.tensor_copy(out=wa116[:], in_=wa1S[:])

    xb16 = cst.tile([49, BC], BF16)
    nc.vector.tensor_copy(out=xb16[:], in_=xTs[:])
    gsp16 = round_r(gspS, [49, 1024], "gsp16")
    ws2b = round_r(ws2S, [64, 64], "ws2b")
    ws3b = round_r(ws3S, [64, 16], "ws3b")
    aps16 = round_r(apsS, [32, 128], "aps16")
    apa16 = round_r(apaS, [32, 128], "apa16")
    asp16 = round_r(aspS, [16, 128], "asp16")
    asc16 = round_r(ascS, [4, 128], "asc16")
    wr2b = round_r(wr2S, [128, 128], "wr2b")
    wr3b = round_r(wr3S, [128, 128], "wr3b")
    wr4b = round_r(wr4S, [128, 1], "wr4b")

    cb_s1 = cbias(C_S1, "cb_s1")
    cb_w = cbias(C_W, "cb_w")
    cb_wn = cbias(-C_W, "cb_wn")
    cb_g = cbias(C_G, "cb_g")
    cb_tiny = cbias(1e-30, "cb_tiny")

    y_d = io["y"]

    st = [dict() for _ in range(NBLK)]

    def stage_a(blk):
        bo = blk * W
        s = st[blk]
        # pair geometry + scalar features
        dc = pm.tile([120, 6, W], F32, tag="dc", name="dc")
        feat = pm.tile([120, 13, W], BF16, tag="feat", name="feat")
        scp = pm.tile([120, 3, W], F32, tag="scp", name="scp")
        s["feat"], s["scp"] = feat, scp
        for i in range(6):
            ps = mm_ps([120, W])
            nc.tensor.matmul(ps[:], gselS[:, i * 120:(i + 1) * 120],
                             xTs[0:48, bo:bo + W], start=True, stop=True)
            nc.vector.tensor_copy(out=dc[:, i, :], in_=ps[:])
            nc.vector.tensor_copy(out=feat[:, 6 + i, :], in_=ps[:])

        r2 = pm.tile([120, W], F32, tag="r2", name="r2")
        sq = pm.tile([120, W], F32, tag="sq", name="sq")
        nc.vector.tensor_tensor(out=r2[:], in0=dc[:, 0, :], in1=dc[:, 0, :], op=OP.mult)
        nc.vector.tensor_tensor(out=sq[:], in0=dc[:, 1, :], in1=dc[:, 1, :], op=OP.mult)
        nc.vector.tensor_tensor(out=r2[:], in0=r2[:], in1=sq[:], op=OP.add)
        nc.vector.tensor_tensor(out=sq[:], in0=dc[:, 2, :], in1=dc[:, 2, :], op=OP.mult)
        nc.vector.tensor_tensor(out=r2[:], in0=r2[:], in1=sq[:], op=OP.add)

        def tmp():
            return pm.tile([120, W], F32, tag="tmp", name="tmp", bufs=4)

        nc.scalar.activation(out=scp[:, 1, :], in_=r2[:], func=AF.Ln,
                             scale=A_S1, bias=cb_s1[:])
        nc.vector.tensor_copy(out=feat[:, 0, :], in_=scp[:, 1, :])
        t1 = tmp()
        nc.scalar.activation(out=t1[:], in_=r2[:], func=AF.Ln, scale=A_S1, bias=cb_w[:])
        t2 = tmp()
        nc.scalar.activation(out=t2[:], in_=t1[:], func=AF.Exp, scale=-1.0)
        nc.vector.tensor_scalar(out=feat[:, 1, :], in0=t2[:], scalar1=-1.0,
                                scalar2=1.0, op0=OP.mult, op1=OP.add)
        t3 = tmp()
        nc.scalar.activation(out=t3[:], in_=r2[:], func=AF.Exp, scale=-A_S1, bias=cb_wn[:])
        t4 = tmp()
        nc.vector.tensor_scalar(out=t4[:], in0=r2[:], scalar1=A_S1, scalar2=C_W,
                                op0=OP.mult, op1=OP.add)
        nc.vector.tensor_tensor(out=feat[:, 2, :], in0=t4[:], in1=t3[:], op=OP.mult)
        for j, g in enumerate((0.25, 1.0, 4.0)):
            nc.scalar.activation(out=feat[:, 3 + j, :], in_=scp[:, 1, :],
                                 func=AF.Exp, scale=-g)
        t5 = tmp()
        nc.scalar.activation(out=t5[:], in_=r2[:], func=AF.Ln, scale=A_G, bias=cb_g[:])
        t6 = tmp()
        nc.scalar.activation(out=t6[:], in_=t5[:], func=AF.Exp, scale=-1.0)
        nc.vector.tensor_scalar(out=scp[:, 0, :], in0=t6[:], scalar1=-1.0,
                                scalar2=1.0, op0=OP.mult, op1=OP.add)
        nc.vector.tensor_copy(out=feat[:, 12, :],
                              in_=pmcS[:, 0:1].to_broadcast([120, W]))
        t7 = tmp()
        nc.scalar.activation(out=t7[:], in_=r2[:], func=AF.Ln, scale=1.0, bias=cb_tiny[:])
        t8 = tmp()
        nc.scalar.activation(out=t8[:], in_=t7[:], func=AF.Exp, scale=0.5)
        t9 = tmp()
        nc.vector.scalar_tensor_tensor(out=t9[:], in0=t7[:], scalar=0.5, in1=t8[:],
                                       op0=OP.mult, op1=OP.subtract)
        nc.scalar.activation(out=scp[:, 2, :], in_=t9[:], func=AF.Exp,
                             scale=1.0, bias=pmcS[:, 1:2])

        # r2_mean numerator
        sq48 = pm.tile([48, W], F32, tag="sq48", name="sq48")
        nc.scalar.square(out=sq48[:], in_=xTs[0:48, bo:bo + W])
        ps_r2m = sm_ps([1, W])
        nc.tensor.matmul(ps_r2m[:], ones48[:], sq48[:], start=True, stop=True)
        scinr = pm.tile([1, 4 * W], F32, tag="scinr", name="scinr")
        s["scinr"] = scinr
        nc.vector.tensor_copy(out=scinr[:, 0:W], in_=ps_r2m[:])

        # single-particle MLP (bf16)
        hs1 = pm1.tile([64, 16 * W], BF16, tag="hs1", name="hs1", bufs=2)
        hs2 = pm1.tile([64, 16 * W], BF16, tag="hs2", name="hs2", bufs=2)
        for half in range(2):
            ps = mm_ps([64, 1024])
            for nl in range(8):
                n = half * 8 + nl
                nc.tensor.matmul(ps[:, nl * W:(nl + 1) * W],
                                 gsp16[:, n * 64:(n + 1) * 64],
                                 xb16[:, bo:bo + W], start=True, stop=True)
            nc.scalar.activation(out=hs1[:, half * 1024:(half + 1) * 1024], in_=ps[:],
                                 func=AF.Gelu)
        for half in range(2):
            ps = mm_ps([64, 1024])
            for ss in range(2):
                nc.tensor.matmul(ps[:, ss * 512:(ss + 1) * 512], ws2b[:],
                                 hs1[:, half * 1024 + ss * 512:half * 1024 + (ss + 1) * 512],
                                 start=True, stop=True)
            nc.scalar.activation(out=hs2[:, half * 1024:(half + 1) * 1024], in_=ps[:],
                                 func=AF.Gelu, bias=bs2S[0:64, :])
        sp16 = pm.tile([16, W], BF16, tag="sp16", name="sp16")
        s["sp16"] = sp16
        spp = pm.tile([16, 2, W], F32, tag="spp", name="spp")
        for half in range(2):
            ps = mm_ps([16, 1024])
            for ss in range(2):
                nc.tensor.matmul(ps[:, ss * 512:(ss + 1) * 512], ws3b[:],
                                 hs2[:, half * 1024 + ss * 512:half * 1024 + (ss + 1) * 512],
                                 start=True, stop=True)
            nc.vector.tensor_reduce(out=spp[:, half, :],
                                    in_=ps[:].rearrange("f (n b) -> f b n", n=8),
                                    axis=AX.X, op=OP.add)
        nc.vector.tensor_tensor(out=sp16[:], in0=spp[:, 0, :], in1=spp[:, 1, :], op=OP.add)

    def stage_b(blk):
        s = st[blk]
        feat = s["feat"]
        raw = pm1.tile([120, 48, W], BF16, tag="raw", name="raw", bufs=2)
        s["raw"] = raw
        for c in range(NCHUNK):
            p0 = c * CH
            pin = pinp.tile([13, FDC], BF16, tag="pin", name="pin")
            dfeat = drp.tile([CH, 13, W], BF16, tag="dfeat", name="dfeat")
            nc.sync.dma_start(out=dfeat[:], in_=feat[p0:p0 + CH, :, :])
            nc.sync.dma_start(
                out=pin[:].rearrange("c (p b) -> c p b", p=CH),
                in_=dfeat[:].rearrange("p c b -> c p b"))

            hcur = pin
            for wS, bS in ((w1s, b1S), (w2s, b2S), (w3s, b3S), (w4s, b4S)):
                hnext = hp.tile([128, FDC], BF16, tag="h", name="h")
                for ss2 in range(2):
                    ps = mm_ps([128, 1536])
                    for ss in range(3):
                        off = ss2 * 1536 + ss * 512
                        nc.tensor.matmul(ps[:, ss * 512:(ss + 1) * 512], wS[:],
                                         hcur[:, off:off + 512], start=True, stop=True)
                    nc.scalar.activation(out=hnext[:, ss2 * 1536:(ss2 + 1) * 1536],
                                         in_=ps[:], func=AF.Gelu, bias=bS[:])
                hcur = hnext
            stg = pinp.tile([128, 1536], BF16, tag="stg", name="stg")
            for ss2 in range(2):
                po = 64 * ss2
                ps5 = mm_ps([64, 1536])
                for ss in range(3):
                    off = ss2 * 1536 + ss * 512
                    nc.tensor.matmul(ps5[:, ss * 512:(ss + 1) * 512], w5s[:],
                                     hcur[:, off:off + 512], start=True, stop=True)
                for ss in range(3):
                    nc.vector.tensor_copy(
                        out=stg[po:po + 64, ss * 512:(ss + 1) * 512],
                        in_=ps5[:, ss * 512:(ss + 1) * 512])
            for ss2 in range(2):
                po = 64 * ss2
                draw = drp.tile([48, 12, W], BF16, tag="draw", name="draw")
                nc.sync.dma_start(out=draw[:], in_=stg[po:po + 48, :])
                nc.sync.dma_start(
                    out=raw[p0 + 12 * ss2:p0 + 12 * ss2 + 12, :, :],
                    in_=draw[:].rearrange("f p b -> p f b"))

    def stage_c(blk):
        s = st[blk]
        raw, scp, scinr = s["raw"], s["scp"], s["scinr"]
        gate_b = pm.tile([120, W], BF16, tag="gate_b", name="gate_b")
        nc.vector.tensor_copy(out=gate_b[:], in_=scp[:, 0, :])
        gate1 = gate_b[:].rearrange("p (f b) -> p f b", f=1)
        emb = pm1.tile([120, 32, W], BF16, tag="emb", name="emb")
        nc.vector.tensor_tensor(out=emb[:], in0=raw[:, 0:32, :],
                                in1=gate1.to_broadcast([120, 32, W]), op=OP.mult)
        ps32 = pm.tile([32, W], BF16, tag="ps32", name="ps32")
        s["ps32"] = ps32
        embf = emb[:].rearrange("p f b -> p (f b)")
        for ss in range(8):
            ps = sm_ps([1, 512])
            nc.tensor.matmul(ps[:], onesr[:], embf[:, ss * 512:(ss + 1) * 512],
                             start=True, stop=True)
            prow = pm.tile([1, 512], BF16, tag="prow", name="prow", bufs=3)
            nc.vector.tensor_copy(out=prow[:], in_=ps[:])
            nc.sync.dma_start(out=ps32[4 * ss:4 * ss + 4, :], in_=prow[:])

        u16g = pm1.tile([120, 16, W], BF16, tag="u16g", name="u16g")
        c1b = cb16[:, 0:16].rearrange("p (f b) -> p f b", b=1).to_broadcast([120, 16, W])
        ba0b = cb16[:, 16:32].rearrange("p (f b) -> p f b", b=1).to_broadcast([120, 16, W])
        nc.vector.tensor_tensor(out=u16g[:], in0=raw[:, 32:48, :], in1=c1b, op=OP.add)
        nc.vector.tensor_tensor(out=u16g[:], in0=u16g[:],
                                in1=gate1.to_broadcast([120, 16, W]), op=OP.mult)
        nc.vector.tensor_tensor(out=u16g[:], in0=u16g[:], in1=ba0b, op=OP.add)
        nc.scalar.activation(out=u16g[:], in_=u16g[:], func=AF.Tanh)
        wab = wa116[:].rearrange("p (f b) -> p f b", b=1).to_broadcast([120, 16, W])
        nc.vector.tensor_tensor(out=u16g[:], in0=u16g[:], in1=wab, op=OP.mult)
        acc = pm.tile([120, W], F32, tag="acc", name="acc")
        nc.vector.tensor_reduce(
            out=acc[:], in_=u16g[:].rearrange("p f b -> p b f"),
            axis=AX.X, op=OP.add)
        sc2 = pm.tile([120, 2, W], F32, tag="sc2", name="sc2")
        nc.scalar.activation(out=sc2[:, 1, :], in_=acc[:], func=AF.Exp)
        nc.vector.tensor_tensor(out=sc2[:, 0, :], in0=scp[:, 0, :], in1=sc2[:, 1, :],
                                op=OP.mult)
        gexp_b = pm.tile([120, W], BF16, tag="gexp_b", name="gexp_b")
        nc.vector.tensor_copy(out=gexp_b[:], in_=sc2[:, 0, :])
        gexp1 = gexp_b[:].rearrange("p (f b) -> p f b", f=1)
        nc.vector.tensor_tensor(out=emb[:], in0=raw[:, 0:32, :],
                                in1=gexp1.to_broadcast([120, 32, W]), op=OP.mult)
        pe32 = pm.tile([32, W], BF16, tag="pe32", name="pe32")
        embxf = emb[:].rearrange("p f b -> p (f b)")
        for ss in range(8):
            ps = sm_ps([1, 512])
            nc.tensor.matmul(ps[:], onesr[:], embxf[:, ss * 512:(ss + 1) * 512],
                             start=True, stop=True)
            prow2 = pm.tile([1, 512], BF16, tag="prow", name="prow2", bufs=3)
            nc.vector.tensor_copy(out=prow2[:], in_=ps[:])
            nc.sync.dma_start(out=pe32[4 * ss:4 * ss + 4, :], in_=prow2[:])

        ps_sc = sm_ps([1, 3 * W])
        nc.tensor.matmul(ps_sc[:], ones120[:],
                         scp[:].rearrange("p c b -> p (c b)"), start=True, stop=True)
        cuspr = pm.tile([1, W], F32, tag="cuspr", name="cuspr")
        s["cuspr"] = cuspr
        nc.vector.tensor_copy(out=scinr[:, 2 * W:3 * W], in_=ps_sc[:, 0:W])
        nc.vector.tensor_copy(out=scinr[:, W:2 * W], in_=ps_sc[:, W:2 * W])
        nc.vector.tensor_copy(out=cuspr[:], in_=ps_sc[:, 2 * W:3 * W])
        ps_sc2 = sm_ps([1, 2 * W])
        nc.tensor.matmul(ps_sc2[:], ones120[:],
                         sc2[:].rearrange("p c b -> p (c b)"), start=True, stop=True)
        rd = pm.tile([1, W], F32, tag="rd", name="rd")
        nc.vector.reciprocal(out=rd[:], in_=ps_sc2[:, W:2 * W])
        nc.vector.tensor_tensor(out=scinr[:, 3 * W:4 * W], in0=ps_sc2[:, 0:W],
                                in1=rd[:], op=OP.mult)
        scinb = pm.tile([1, 4 * W], BF16, tag="scinb", name="scinb")
        nc.vector.tensor_copy(out=scinb[:], in_=scinr[:])
        scin = pm.tile([4, W], BF16, tag="scin", name="scin")
        s["scin"] = scin
        nc.sync.dma_start(out=scin[:], in_=scinb[:])
        ps_rb = sm_ps([32, W])
        nc.tensor.matmul(ps_rb[:], ones_row[:], rd[:], start=True, stop=True)
        attn32 = pm.tile([32, W], BF16, tag="attn32", name="attn32")
        s["attn32"] = attn32
        nc.vector.tensor_tensor(out=attn32[:], in0=pe32[:], in1=ps_rb[:], op=OP.mult)

    def stage_d(blk):
        bo = blk * W
        s = st[blk]
        psR = mm_ps([128, W])
        nc.tensor.matmul(psR[:], aps16[:], s["ps32"][:], start=True, stop=False)
        nc.tensor.matmul(psR[:], apa16[:], s["attn32"][:], start=False, stop=False)
        nc.tensor.matmul(psR[:], asp16[:], s["sp16"][:], start=False, stop=False)
        nc.tensor.matmul(psR[:], asc16[:], s["scin"][:], start=False, stop=True)
        h1R = pm.tile([128, W], BF16, tag="h1R", name="h1R")
        nc.scalar.activation(out=h1R[:], in_=psR[:], func=AF.Gelu, bias=br1S[:])
        psR2 = mm_ps([128, W])
        nc.tensor.matmul(psR2[:], wr2b[:], h1R[:], start=True, stop=True)
        h2R = pm.tile([128, W], BF16, tag="h2R", name="h2R")
        nc.scalar.activation(out=h2R[:], in_=psR2[:], func=AF.Gelu, bias=br2S[:])
        psR3 = mm_ps([128, W])
        nc.tensor.matmul(psR3[:], wr3b[:], h2R[:], start=True, stop=True)
        h3R = pm.tile([128, W], BF16, tag="h3R", name="h3R")
        nc.scalar.activation(out=h3R[:], in_=psR3[:], func=AF.Gelu, bias=br3S[:])
        psL4 = sm_ps([1, W])
        nc.tensor.matmul(psL4[:], wr4b[:], h3R[:], start=True, stop=True)
        ys = pm.tile([1, W], F32, tag="ys", name="ys")
        nc.vector.scalar_tensor_tensor(out=ys[:], in0=psL4[:], scalar=br4S[0:1, 0:1],
                                       in1=s["cuspr"][:], op0=OP.add, op1=OP.add)
        nc.sync.dma_start(out=y_d[bo:bo + W, 0:1], in_=ys[:])

    # software-pipelined emission: A(b+1)/B(b+1) land between B(b) and C(b)/D(b)
    stage_a(0)
    stage_b(0)
    stage_a(1)
    stage_c(0)
    stage_b(1)
    stage_a(2)
    stage_d(0)
    stage_c(1)
    stage_b(2)
    stage_a(3)
    stage_d(1)
    stage_c(2)
    stage_b(3)
    stage_d(2)
    stage_c(3)
    stage_d(3)


def build():
    nc = bacc.Bacc("TRN2", target_bir_lowering=False, debug=False,
                   num_devices=NCORES)
    io = {}

    def din(name, shape, dtype=F32):
        io[name] = nc.dram_tensor(name, shape, dtype, kind="ExternalInput").ap()

    din("xT", [49, BC])
    din("gsel", [48, 720])
    din("w1", [13, 128])
    din("w2", [128, 128])
    din("w3", [128, 128])
    din("w4", [128, 128])
    din("w5p", [128, 64])
    for nm in ("b1", "b2", "b3", "b4"):
        din(nm, [128, 1])
    din("wa1bc", [120, 16])
    din("c1ba0", [120, 32])
    din("pmc", [120, 2])
    din("gsp", [49, 1024])
    din("ws2", [64, 64])
    din("bs2", [64, 1])
    din("ws3", [64, 16])
    din("aps", [32, 128])
    din("apa", [32, 128])
    din("asp", [16, 128])
    din("asc", [4, 128])
    din("br1", [128, 1])
    din("wr2", [128, 128])
    din("br2", [128, 1])
    din("wr3", [128, 128])
    din("br3", [128, 1])
    din("wr4", [128, 1])
    din("br4", [1, 1])
    io["y"] = nc.dram_tensor("y", [BC, 1], F32, kind="ExternalOutput").ap()

    with tile.TileContext(nc) as tc, ExitStack() as ctx:
        _emit(tc, ctx, io)
    nc.compile()
    return nc


_NC = None


def _get_nc():
    global _NC
    if _NC is None:
        _NC = build()
    return _NC


def host_inputs(x, pair_params, attn_params, sp_params, readout_params):
    """Build shared weight map + per-core input maps."""
    a = lambda v: np.asarray(v, np.float32)
    x = a(x)
    (W1, b1), (W2, b2), (W3, b3), (W4, b4), (W5, b5) = \
        [(a(w), a(b)) for w, b in pair_params]
    (Wa0, ba0), (Wa1, ba1) = [(a(w), a(b)) for w, b in attn_params]
    (Ws1, bs1), (Ws2, bs2), (Ws3, bs3) = [(a(w), a(b)) for w, b in sp_params]
    (Wr1, br1), (Wr2, br2), (Wr3, br3), (Wr4, br4) = \
        [(a(w), a(b)) for w, b in readout_params]

    W5p = np.concatenate([W5, W5 @ Wa0,
                          np.zeros((128, 16), np.float32)], axis=1)  # [128, 64]
    c1 = b5 @ Wa0                                          # [16]
    A_ps = Wr1[0:32] + Wr1[32:64] / P
    A_pa = Wr1[64:96]
    A_sp = Wr1[96:112] + Wr1[112:128] / N
    asc = np.stack([Wr1[128] / 48.0, Wr1[129] / P,
                    A_ps.T @ b5, A_pa.T @ b5]).astype(np.float32)  # [4, 128]
    br1_eff = br1 + N * (A_sp.T @ bs3)

    gsp = np.zeros((49, 16 * 64), np.float32)
    for n in range(N):
        for d in range(3):
            gsp[n * 3 + d, n * 64:(n + 1) * 64] = Ws1[d]
        gsp[48, n * 64:(n + 1) * 64] = Ws1[3] * SPIN_F[n] + bs1

    col = lambda v: np.ascontiguousarray(v.reshape(-1, 1), dtype=np.float32)
    shared = dict(
        gsel=_gsel_const(),
        w1=W1, w2=W2, w3=W3, w4=W4, w5p=W5p,
        b1=col(b1), b2=col(b2), b3=col(b3), b4=col(b4),
        wa1bc=np.tile(Wa1.reshape(1, 16), (P, 1)).astype(np.float32),
        c1ba0=np.tile(np.concatenate([c1, ba0]).reshape(1, 32), (P, 1)).astype(np.float32),
        pmc=np.stack([SPIN_MATCH, LNGAMMA], axis=1).astype(np.float32),
        gsp=gsp, ws2=Ws2, bs2=col(bs2), ws3=Ws3,
        aps=A_ps, apa=A_pa, asp=A_sp, asc=asc,
        br1=col(br1_eff), wr2=Wr2, br2=col(br2),
        wr3=Wr3, br3=col(br3), wr4=Wr4.reshape(128, 1), br4=br4.reshape(1, 1),
    )
    shared = {k: np.ascontiguousarray(v, dtype=np.float32)
              for k, v in shared.items()}
    in_maps = []
    for core in range(NCORES):
        xc = x[core * BC:(core + 1) * BC]                     # [BC, 16, 3]
        xT = np.concatenate([xc.transpose(1, 2, 0).reshape(48, BC),
                             np.ones((1, BC), np.float32)], axis=0)
        in_maps.append({**shared, "xT": np.ascontiguousarray(xT, np.float32)})
    return in_maps


def run(in_maps, trace=False, trace_kwargs=None):
    if trace:
        _install_ntff_hook()
    return run_bass_kernel_spmd(_get_nc(), in_maps, list(range(NCORES)),
                                trace=trace, **(trace_kwargs or {}))


def _install_ntff_hook():
    if "antenv.axon_hooks" in sys.modules:
        return
    m = types.ModuleType("antenv.axon_hooks")
    m._hook = None
    m.set_axon_ntff_profile_hook = lambda h: setattr(m, "_hook", h)
    m.get_axon_ntff_profile_hook = lambda: m._hook
    sys.modules["antenv.axon_hooks"] = m
    import antenv
    antenv.axon_hooks = m
    from trn_agent_boot.trn_boot import _ntff_profile_via_ctypes
    m.set_axon_ntff_profile_hook(_ntff_profile_via_ctypes("/opt/axon/libaxon_pjrt.so"))


def kernel(x, pair_params, attn_params, sp_params, readout_params):
    in_maps = host_inputs(x, pair_params, attn_params, sp_params, readout_params)
    res = run(in_maps)
    out = np.concatenate([res.results[i]["y"] for i in range(NCORES)], axis=0)
    return np.ascontiguousarray(out, dtype=np.float32)


# revision 26
# speedup vs baseline: 1.1870x; 1.1870x over previous
"""DeepSet Jastrow factor kernel for Trainium2 (8 NeuronCores, data parallel).

Self-contained: builds a Bass/Tile kernel once (cached), shards the batch
across 8 cores, runs via run_bass_kernel_spmd, gathers the output.
"""

import sys
import types
from contextlib import ExitStack

sys.path.insert(0, "/opt/trn_rl_repo")

import numpy as np

import concourse.bass as bass
import concourse.tile as tile
from concourse import bacc, mybir
from concourse.bass_utils import run_bass_kernel_spmd

F32 = mybir.dt.float32
F32R = mybir.dt.float32r
BF16 = mybir.dt.bfloat16
AF = mybir.ActivationFunctionType
OP = mybir.AluOpType
AX = mybir.AxisListType

# ---------------- problem constants (hardcoded per spec) ----------------
B, N, D = 4096, 16, 3
P = N * (N - 1) // 2  # 120
NCORES = 8
BC = B // NCORES  # 512 samples per core
W = 128           # batch block width
NBLK = BC // W    # 4 blocks per core
CH = 24           # pairs per MLP chunk
NCHUNK = P // CH  # 5
FDC = CH * W      # 3072 tokens per chunk

F32EPS = float(np.finfo(np.float32).eps)
EPS2 = np.float64(0.04)        # EPS_FEAT^2, EPS_FEAT = 0.2
G2 = np.float64(0.09)          # GATE_R^2, GATE_R = 0.3

A_S1 = float(1.0 / EPS2)                      # 25
C_S1 = float((F32EPS + 2 * EPS2) / EPS2)      # ln(1+rt2) arg
C_W = float((F32EPS + EPS2) / EPS2)           # ln(rt2) arg
A_G = float(1.0 / G2)
C_G = float((F32EPS + G2) / G2)

IDX_I, IDX_J = np.triu_indices(N, k=1)
_spin = np.concatenate([np.zeros(N // 2), np.ones(N - N // 2)])
SPIN_MATCH = (_spin[IDX_I] == _spin[IDX_J]).astype(np.float32)
GAMMA = SPIN_MATCH * (1.0 / (D + 1)) + (1.0 - SPIN_MATCH) * (1.0 / (D - 1))
LNGAMMA = np.log(GAMMA).astype(np.float32)
SPIN_F = _spin.astype(np.float32)


def _gsel_const():
    """[48, 720] six stacked selection matrices: diff d0..2, cent d0..2."""
    G = np.zeros((6, 48, P), np.float32)
    for p in range(P):
        i, j = int(IDX_I[p]), int(IDX_J[p])
        for d in range(3):
            G[d, i * 3 + d, p] += 1.0
            G[d, j * 3 + d, p] -= 1.0
            G[3 + d, i * 3 + d, p] += 0.5
            G[3 + d, j * 3 + d, p] += 0.5
    return np.concatenate(list(G), axis=1)  # [48, 720]


# ---------------- device program ----------------

def _emit(tc, ctx, io):
    nc = tc.nc
    cst = ctx.enter_context(tc.tile_pool(name="cst", bufs=1))
    pm = ctx.enter_context(tc.tile_pool(name="pm", bufs=2))
    pm1 = ctx.enter_context(tc.tile_pool(name="pm1", bufs=1))
    pinp = ctx.enter_context(tc.tile_pool(name="pinp", bufs=2))
    hp = ctx.enter_context(tc.tile_pool(name="hp", bufs=2))
    drp = ctx.enter_context(tc.tile_pool(name="drp", bufs=2, space="DRAM"))
    mmps = ctx.enter_context(tc.tile_pool(name="mmps", bufs=2, space="PSUM"))
    smps = ctx.enter_context(tc.tile_pool(name="smps", bufs=2, space="PSUM"))

    def mm_ps(shape):
        return mmps.tile(shape, F32, tag="mm", name="mmtile")

    def sm_ps(shape):
        return smps.tile(shape, F32, tag="sm", name="smtile")

    # ---- load constants/weights to SBUF ----
    def load(name, shape, dtype=F32):
        t = cst.tile(shape, dtype, name=name)
        nc.sync.dma_start(out=t[:], in_=io[name][:])
        return t

    xTs = load("xT", [49, BC])
    gselS = load("gsel", [48, 720])
    w1f = load("w1", [13, 128])
    w2f = load("w2", [128, 128])
    w3f = load("w3", [128, 128])
    w4f = load("w4", [128, 128])
    w5f = load("w5p", [128, 64])
    b1S = load("b1", [128, 1])
    b2S = load("b2", [128, 1])
    b3S = load("b3", [128, 1])
    b4S = load("b4", [128, 1])
    wa1S = load("wa1bc", [120, 16])
    cbS = load("c1ba0", [120, 32])
    pmcS = load("pmc", [120, 2])
    gspS = load("gsp", [49, 1024])
    ws2S = load("ws2", [64, 64])
    bs2S = load("bs2", [64, 1])
    ws3S = load("ws3", [64, 16])
    apsS = load("aps", [32, 128])
    apaS = load("apa", [32, 128])
    aspS = load("asp", [16, 128])
    ascS = load("asc", [4, 128])
    br1S = load("br1", [128, 1])
    wr2S = load("wr2", [128, 128])
    br2S = load("br2", [128, 1])
    wr3S = load("wr3", [128, 128])
    br3S = load("br3", [128, 1])
    wr4S = load("wr4", [128, 1])
    br4S = load("br4", [1, 1])

    # fp32r-rounded pair-MLP weights
    def round_r(src, shape, name):
        t = cst.tile(shape, BF16, name=name)
        nc.vector.tensor_copy(out=t[:], in_=src[:])
        return t

    w1s = round_r(w1f, [13, 128], "w1s")
    w2s = round_r(w2f, [128, 128], "w2s")
    w3s = round_r(w3f, [128, 128], "w3s")
    w4s = round_r(w4f, [128, 128], "w4s")
    w5s = round_r(w5f, [128, 64], "w5s")

    ones120 = cst.tile([120, 1], F32)
    nc.vector.memset(ones120[:], 1.0)
    ones48 = cst.tile([48, 1], F32)
    nc.vector.memset(ones48[:], 1.0)
    onesr = cst.tile([120, 1], BF16)
    nc.vector.tensor_copy(out=onesr[:], in_=ones120[:])

    def cbias(val, name):
        t = cst.tile([120, 1], F32, name=name)
        nc.vector.memset(t[:], val)
        return t

    ones_row = cst.tile([1, 32], F32)
    nc.vector.memset(ones_row[:], 1.0)
    cb16 = cst.tile([120, 32], BF16)
    nc.vector.tensor_copy(out=cb16[:], in_=cbS[:])
    wa116 = cst.tile([120, 16], BF16)
    nc.vector.tensor_copy(out=wa116[:], in_=wa1S[:])

    xb16 = cst.tile([49, BC], BF16)
    nc.vector.tensor_copy(out=xb16[:], in_=xTs[:])
    gsp16 = round_r(gspS, [49, 1024], "gsp16")
    ws2b = round_r(ws2S, [64, 64], "ws2b")
    ws3b = round_r(ws3S, [64, 16], "ws3b")
    aps16 = round_r(apsS, [32, 128], "aps16")
    apa16 = round_r(apaS, [32, 128], "apa16")
    asp16 = round_r(aspS, [16, 128], "asp16")
    asc16 = round_r(ascS, [4, 128], "asc16")
    wr2b = round_r(wr2S, [128, 128], "wr2b")
    wr3b = round_r(wr3S, [128, 128], "wr3b")
    wr4b = round_r(wr4S, [128, 1], "wr4b")

    cb_s1 = cbias(C_S1, "cb_s1")
    cb_w = cbias(C_W, "cb_w")
    cb_wn = cbias(-C_W, "cb_wn")
    cb_g = cbias(C_G, "cb_g")
    cb_tiny = cbias(1e-30, "cb_tiny")

    y_d = io["y"]

    st = [dict() for _ in range(NBLK)]

    def stage_a(blk):
        bo = blk * W
        s = st[blk]
        # pair geometry + scalar features
        dc = pm.tile([120, 6, W], F32, tag="dc", name="dc")
        feat = pm.tile([120, 13, W], BF16, tag="feat", name="feat", bufs=4)
        scp = pm.tile([120, 3, W], F32, tag="scp", name="scp", bufs=4)
        s["feat"], s["scp"] = feat, scp
        for i in range(6):
            ps = mm_ps([120, W])
            nc.tensor.matmul(ps[:], gselS[:, i * 120:(i + 1) * 120],
                             xTs[0:48, bo:bo + W], start=True, stop=True)
            nc.vector.tensor_copy(out=dc[:, i, :], in_=ps[:])
            nc.vector.tensor_copy(out=feat[:, 6 + i, :], in_=ps[:])

        r2 = pm.tile([120, W], F32, tag="r2", name="r2")
        sq = pm.tile([120, W], F32, tag="sq", name="sq")
        nc.vector.tensor_tensor(out=r2[:], in0=dc[:, 0, :], in1=dc[:, 0, :], op=OP.mult)
        nc.vector.tensor_tensor(out=sq[:], in0=dc[:, 1, :], in1=dc[:, 1, :], op=OP.mult)
        nc.vector.tensor_tensor(out=r2[:], in0=r2[:], in1=sq[:], op=OP.add)
        nc.vector.tensor_tensor(out=sq[:], in0=dc[:, 2, :], in1=dc[:, 2, :], op=OP.mult)
        nc.vector.tensor_tensor(out=r2[:], in0=r2[:], in1=sq[:], op=OP.add)

        def tmp():
            return pm.tile([120, W], F32, tag="tmp", name="tmp", bufs=4)

        nc.scalar.activation(out=scp[:, 1, :], in_=r2[:], func=AF.Ln,
                             scale=A_S1, bias=cb_s1[:])
        nc.vector.tensor_copy(out=feat[:, 0, :], in_=scp[:, 1, :])
        t1 = tmp()
        nc.scalar.activation(out=t1[:], in_=r2[:], func=AF.Ln, scale=A_S1, bias=cb_w[:])
        t2 = tmp()
        nc.scalar.activation(out=t2[:], in_=t1[:], func=AF.Exp, scale=-1.0)
        nc.vector.tensor_scalar(out=feat[:, 1, :], in0=t2[:], scalar1=-1.0,
                                scalar2=1.0, op0=OP.mult, op1=OP.add)
        t3 = tmp()
        nc.scalar.activation(out=t3[:], in_=r2[:], func=AF.Exp, scale=-A_S1, bias=cb_wn[:])
        t4 = tmp()
        nc.vector.tensor_scalar(out=t4[:], in0=r2[:], scalar1=A_S1, scalar2=C_W,
                                op0=OP.mult, op1=OP.add)
        nc.vector.tensor_tensor(out=feat[:, 2, :], in0=t4[:], in1=t3[:], op=OP.mult)
        for j, g in enumerate((0.25, 1.0, 4.0)):
            nc.scalar.activation(out=feat[:, 3 + j, :], in_=scp[:, 1, :],
                                 func=AF.Exp, scale=-g)
        t5 = tmp()
        nc.scalar.activation(out=t5[:], in_=r2[:], func=AF.Ln, scale=A_G, bias=cb_g[:])
        t6 = tmp()
        nc.scalar.activation(out=t6[:], in_=t5[:], func=AF.Exp, scale=-1.0)
        nc.vector.tensor_scalar(out=scp[:, 0, :], in0=t6[:], scalar1=-1.0,
                                scalar2=1.0, op0=OP.mult, op1=OP.add)
        nc.vector.tensor_copy(out=feat[:, 12, :],
                              in_=pmcS[:, 0:1].to_broadcast([120, W]))
        t7 = tmp()
        nc.scalar.activation(out=t7[:], in_=r2[:], func=AF.Ln, scale=1.0, bias=cb_tiny[:])
        t8 = tmp()
        nc.scalar.activation(out=t8[:], in_=t7[:], func=AF.Exp, scale=0.5)
        t9 = tmp()
        nc.vector.scalar_tensor_tensor(out=t9[:], in0=t7[:], scalar=0.5, in1=t8[:],
                                       op0=OP.mult, op1=OP.subtract)
        nc.scalar.activation(out=scp[:, 2, :], in_=t9[:], func=AF.Exp,
                             scale=1.0, bias=pmcS[:, 1:2])

        # r2_mean numerator
        sq48 = pm.tile([48, W], F32, tag="sq48", name="sq48")
        nc.scalar.square(out=sq48[:], in_=xTs[0:48, bo:bo + W])
        ps_r2m = sm_ps([1, W])
        nc.tensor.matmul(ps_r2m[:], ones48[:], sq48[:], start=True, stop=True)
        scinr = pm.tile([1, 4 * W], F32, tag="scinr", name="scinr", bufs=4)
        s["scinr"] = scinr
        nc.vector.tensor_copy(out=scinr[:, 0:W], in_=ps_r2m[:])

    def stage_sp(blk):
        bo = blk * W
        s = st[blk]
        hs1 = pm1.tile([64, 16 * W], BF16, tag="hs1", name="hs1", bufs=2)
        hs2 = pm1.tile([64, 16 * W], BF16, tag="hs2", name="hs2", bufs=2)
        for half in range(2):
            ps = mm_ps([64, 1024])
            for nl in range(8):
                n = half * 8 + nl
                nc.tensor.matmul(ps[:, nl * W:(nl + 1) * W],
                                 gsp16[:, n * 64:(n + 1) * 64],
                                 xb16[:, bo:bo + W], start=True, stop=True)
            nc.scalar.activation(out=hs1[:, half * 1024:(half + 1) * 1024], in_=ps[:],
                                 func=AF.Gelu)
        for half in range(2):
            ps = mm_ps([64, 1024])
            for ss in range(2):
                nc.tensor.matmul(ps[:, ss * 512:(ss + 1) * 512], ws2b[:],
                                 hs1[:, half * 1024 + ss * 512:half * 1024 + (ss + 1) * 512],
                                 start=True, stop=True)
            nc.scalar.activation(out=hs2[:, half * 1024:(half + 1) * 1024], in_=ps[:],
                                 func=AF.Gelu, bias=bs2S[0:64, :])
        sp16 = pm.tile([16, W], BF16, tag="sp16", name="sp16", bufs=4)
        s["sp16"] = sp16
        spp = pm.tile([16, 2, W], F32, tag="spp", name="spp")
        for half in range(2):
            ps = mm_ps([16, 1024])
            for ss in range(2):
                nc.tensor.matmul(ps[:, ss * 512:(ss + 1) * 512], ws3b[:],
                                 hs2[:, half * 1024 + ss * 512:half * 1024 + (ss + 1) * 512],
                                 start=True, stop=True)
            nc.vector.tensor_reduce(out=spp[:, half, :],
                                    in_=ps[:].rearrange("f (n b) -> f b n", n=8),
                                    axis=AX.X, op=OP.add)
        nc.vector.tensor_tensor(out=sp16[:], in0=spp[:, 0, :], in1=spp[:, 1, :], op=OP.add)

    def stage_b(blk):
        s = st[blk]
        feat = s["feat"]
        raw = pm1.tile([120, 48, W], BF16, tag="raw", name="raw", bufs=2)
        s["raw"] = raw
        for c in range(NCHUNK):
            p0 = c * CH
            pin = pinp.tile([13, FDC], BF16, tag="pin", name="pin")
            dfeat = drp.tile([CH, 13, W], BF16, tag="dfeat", name="dfeat")
            nc.sync.dma_start(out=dfeat[:], in_=feat[p0:p0 + CH, :, :])
            nc.sync.dma_start(
                out=pin[:].rearrange("c (p b) -> c p b", p=CH),
                in_=dfeat[:].rearrange("p c b -> c p b"))

            hcur = pin
            for wS, bS in ((w1s, b1S), (w2s, b2S), (w3s, b3S), (w4s, b4S)):
                hnext = hp.tile([128, FDC], BF16, tag="h", name="h")
                for ss2 in range(2):
                    ps = mm_ps([128, 1536])
                    for ss in range(3):
                        off = ss2 * 1536 + ss * 512
                        nc.tensor.matmul(ps[:, ss * 512:(ss + 1) * 512], wS[:],
                                         hcur[:, off:off + 512], start=True, stop=True)
                    nc.scalar.activation(out=hnext[:, ss2 * 1536:(ss2 + 1) * 1536],
                                         in_=ps[:], func=AF.Gelu, bias=bS[:])
                hcur = hnext
            stg = pinp.tile([128, 1536], BF16, tag="stg", name="stg")
            for ss2 in range(2):
                po = 64 * ss2
                ps5 = mm_ps([64, 1536])
                for ss in range(3):
                    off = ss2 * 1536 + ss * 512
                    nc.tensor.matmul(ps5[:, ss * 512:(ss + 1) * 512], w5s[:],
                                     hcur[:, off:off + 512], start=True, stop=True)
                for ss in range(3):
                    nc.vector.tensor_copy(
                        out=stg[po:po + 64, ss * 512:(ss + 1) * 512],
                        in_=ps5[:, ss * 512:(ss + 1) * 512])
            for ss2 in range(2):
                po = 64 * ss2
                draw = drp.tile([48, 12, W], BF16, tag="draw", name="draw")
                nc.sync.dma_start(out=draw[:], in_=stg[po:po + 48, :])
                nc.sync.dma_start(
                    out=raw[p0 + 12 * ss2:p0 + 12 * ss2 + 12, :, :],
                    in_=draw[:].rearrange("f p b -> p f b"))

    def stage_c(blk):
        s = st[blk]
        raw, scp, scinr = s["raw"], s["scp"], s["scinr"]
        gate_b = pm.tile([120, W], BF16, tag="gate_b", name="gate_b")
        nc.vector.tensor_copy(out=gate_b[:], in_=scp[:, 0, :])
        gate1 = gate_b[:].rearrange("p (f b) -> p f b", f=1)
        emb = pm1.tile([120, 32, W], BF16, tag="emb", name="emb")
        nc.vector.tensor_tensor(out=emb[:], in0=raw[:, 0:32, :],
                                in1=gate1.to_broadcast([120, 32, W]), op=OP.mult)
        ps32 = pm.tile([32, W], BF16, tag="ps32", name="ps32")
        s["ps32"] = ps32
        embf = emb[:].rearrange("p f b -> p (f b)")
        for ss in range(8):
            ps = sm_ps([1, 512])
            nc.tensor.matmul(ps[:], onesr[:], embf[:, ss * 512:(ss + 1) * 512],
                             start=True, stop=True)
            prow = pm.tile([1, 512], BF16, tag="prow", name="prow", bufs=3)
            nc.vector.tensor_copy(out=prow[:], in_=ps[:])
            nc.sync.dma_start(out=ps32[4 * ss:4 * ss + 4, :], in_=prow[:])

        u16g = pm1.tile([120, 16, W], BF16, tag="u16g", name="u16g")
        c1b = cb16[:, 0:16].rearrange("p (f b) -> p f b", b=1).to_broadcast([120, 16, W])
        ba0b = cb16[:, 16:32].rearrange("p (f b) -> p f b", b=1).to_broadcast([120, 16, W])
        nc.vector.tensor_tensor(out=u16g[:], in0=raw[:, 32:48, :], in1=c1b, op=OP.add)
        nc.vector.tensor_tensor(out=u16g[:], in0=u16g[:],
                                in1=gate1.to_broadcast([120, 16, W]), op=OP.mult)
        nc.vector.tensor_tensor(out=u16g[:], in0=u16g[:], in1=ba0b, op=OP.add)
        nc.scalar.activation(out=u16g[:], in_=u16g[:], func=AF.Tanh)
        wab = wa116[:].rearrange("p (f b) -> p f b", b=1).to_broadcast([120, 16, W])
        nc.vector.tensor_tensor(out=u16g[:], in0=u16g[:], in1=wab, op=OP.mult)
        acc = pm.tile([120, W], F32, tag="acc", name="acc")
        nc.vector.tensor_reduce(
            out=acc[:], in_=u16g[:].rearrange("p f b -> p b f"),
            axis=AX.X, op=OP.add)
        sc2 = pm.tile([120, 2, W], F32, tag="sc2", name="sc2")
        nc.scalar.activation(out=sc2[:, 1, :], in_=acc[:], func=AF.Exp)
        nc.vector.tensor_tensor(out=sc2[:, 0, :], in0=scp[:, 0, :], in1=sc2[:, 1, :],
                                op=OP.mult)
        gexp_b = pm.tile([120, W], BF16, tag="gexp_b", name="gexp_b")
        nc.vector.tensor_copy(out=gexp_b[:], in_=sc2[:, 0, :])
        gexp1 = gexp_b[:].rearrange("p (f b) -> p f b", f=1)
        nc.vector.tensor_tensor(out=emb[:], in0=raw[:, 0:32, :],
                                in1=gexp1.to_broadcast([120, 32, W]), op=OP.mult)
        pe32 = pm.tile([32, W], BF16, tag="pe32", name="pe32")
        embxf = emb[:].rearrange("p f b -> p (f b)")
        for ss in range(8):
            ps = sm_ps([1, 512])
            nc.tensor.matmul(ps[:], onesr[:], embxf[:, ss * 512:(ss + 1) * 512],
                             start=True, stop=True)
            prow2 = pm.tile([1, 512], BF16, tag="prow", name="prow2", bufs=3)
            nc.vector.tensor_copy(out=prow2[:], in_=ps[:])
            nc.sync.dma_start(out=pe32[4 * ss:4 * ss + 4, :], in_=prow2[:])

        ps_sc = sm_ps([1, 3 * W])
        nc.tensor.matmul(ps_sc[:], ones120[:],
                         scp[:].rearrange("p c b -> p (c b)"), start=True, stop=True)
        cuspr = pm.tile([1, W], F32, tag="cuspr", name="cuspr", bufs=4)
        s["cuspr"] = cuspr
        nc.vector.tensor_copy(out=scinr[:, 2 * W:3 * W], in_=ps_sc[:, 0:W])
        nc.vector.tensor_copy(out=scinr[:, W:2 * W], in_=ps_sc[:, W:2 * W])
        nc.vector.tensor_copy(out=cuspr[:], in_=ps_sc[:, 2 * W:3 * W])
        ps_sc2 = sm_ps([1, 2 * W])
        nc.tensor.matmul(ps_sc2[:], ones120[:],
                         sc2[:].rearrange("p c b -> p (c b)"), start=True, stop=True)
        rd = pm.tile([1, W], F32, tag="rd", name="rd")
        nc.vector.reciprocal(out=rd[:], in_=ps_sc2[:, W:2 * W])
        nc.vector.tensor_tensor(out=scinr[:, 3 * W:4 * W], in0=ps_sc2[:, 0:W],
                                in1=rd[:], op=OP.mult)
        scinb = pm.tile([1, 4 * W], BF16, tag="scinb", name="scinb")
        nc.vector.tensor_copy(out=scinb[:], in_=scinr[:])
        scin = pm.tile([4, W], BF16, tag="scin", name="scin")
        s["scin"] = scin
        nc.sync.dma_start(out=scin[:], in_=scinb[:])
        ps_rb = sm_ps([32, W])
        nc.tensor.matmul(ps_rb[:], ones_row[:], rd[:], start=True, stop=True)
        attn32 = pm.tile([32, W], BF16, tag="attn32", name="attn32")
        s["attn32"] = attn32
        nc.vector.tensor_tensor(out=attn32[:], in0=pe32[:], in1=ps_rb[:], op=OP.mult)

    def stage_d(blk):
        bo = blk * W
        s = st[blk]
        psR = mm_ps([128, W])
        nc.tensor.matmul(psR[:], aps16[:], s["ps32"][:], start=True, stop=False)
        nc.tensor.matmul(psR[:], apa16[:], s["attn32"][:], start=False, stop=False)
        nc.tensor.matmul(psR[:], asp16[:], s["sp16"][:], start=False, stop=False)
        nc.tensor.matmul(psR[:], asc16[:], s["scin"][:], start=False, stop=True)
        h1R = pm.tile([128, W], BF16, tag="h1R", name="h1R")
        nc.scalar.activation(out=h1R[:], in_=psR[:], func=AF.Gelu, bias=br1S[:])
        psR2 = mm_ps([128, W])
        nc.tensor.matmul(psR2[:], wr2b[:], h1R[:], start=True, stop=True)
        h2R = pm.tile([128, W], BF16, tag="h2R", name="h2R")
        nc.scalar.activation(out=h2R[:], in_=psR2[:], func=AF.Gelu, bias=br2S[:])
        psR3 = mm_ps([128, W])
        nc.tensor.matmul(psR3[:], wr3b[:], h2R[:], start=True, stop=True)
        h3R = pm.tile([128, W], BF16, tag="h3R", name="h3R")
        nc.scalar.activation(out=h3R[:], in_=psR3[:], func=AF.Gelu, bias=br3S[:])
        psL4 = sm_ps([1, W])
        nc.tensor.matmul(psL4[:], wr4b[:], h3R[:], start=True, stop=True)
        ys = pm.tile([1, W], F32, tag="ys", name="ys")
        nc.vector.scalar_tensor_tensor(out=ys[:], in0=psL4[:], scalar=br4S[0:1, 0:1],
                                       in1=s["cuspr"][:], op0=OP.add, op1=OP.add)
        nc.sync.dma_start(out=y_d[bo:bo + W, 0:1], in_=ys[:])

    # features for every block first (one ln/exp table era), then the
    # gelu-heavy MLP stages pipelined with attention/readout
    for b in range(NBLK):
        stage_a(b)
    stage_b(0)
    stage_sp(0)
    stage_b(1)
    stage_sp(1)
    stage_c(0)
    stage_b(2)
    stage_sp(2)
    stage_d(0)
    stage_c(1)
    stage_b(3)
    stage_sp(3)
    stage_d(1)
    stage_c(2)
    stage_d(2)
    stage_c(3)
    stage_d(3)


def build():
    nc = bacc.Bacc("TRN2", target_bir_lowering=False, debug=False,
                   num_devices=NCORES)
    io = {}

    def din(name, shape, dtype=F32):
        io[name] = nc.dram_tensor(name, shape, dtype, kind="ExternalInput").ap()

    din("xT", [49, BC])
    din("gsel", [48, 720])
    din("w1", [13, 128])
    din("w2", [128, 128])
    din("w3", [128, 128])
    din("w4", [128, 128])
    din("w5p", [128, 64])
    for nm in ("b1", "b2", "b3", "b4"):
        din(nm, [128, 1])
    din("wa1bc", [120, 16])
    din("c1ba0", [120, 32])
    din("pmc", [120, 2])
    din("gsp", [49, 1024])
    din("ws2", [64, 64])
    din("bs2", [64, 1])
    din("ws3", [64, 16])
    din("aps", [32, 128])
    din("apa", [32, 128])
    din("asp", [16, 128])
    din("asc", [4, 128])
    din("br1", [128, 1])
    din("wr2", [128, 128])
    din("br2", [128, 1])
    din("wr3", [128, 128])
    din("br3", [128, 1])
    din("wr4", [128, 1])
    din("br4", [1, 1])
    io["y"] = nc.dram_tensor("y", [BC, 1], F32, kind="ExternalOutput").ap()

    with tile.TileContext(nc) as tc, ExitStack() as ctx:
        _emit(tc, ctx, io)
    nc.compile()
    return nc


_NC = None


def _get_nc():
    global _NC
    if _NC is None:
        _NC = build()
    return _NC


def host_inputs(x, pair_params, attn_params, sp_params, readout_params):
    """Build shared weight map + per-core input maps."""
    a = lambda v: np.asarray(v, np.float32)
    x = a(x)
    (W1, b1), (W2, b2), (W3, b3), (W4, b4), (W5, b5) = \
        [(a(w), a(b)) for w, b in pair_params]
    (Wa0, ba0), (Wa1, ba1) = [(a(w), a(b)) for w, b in attn_params]
    (Ws1, bs1), (Ws2, bs2), (Ws3, bs3) = [(a(w), a(b)) for w, b in sp_params]
    (Wr1, br1), (Wr2, br2), (Wr3, br3), (Wr4, br4) = \
        [(a(w), a(b)) for w, b in readout_params]

    W5p = np.concatenate([W5, W5 @ Wa0,
                          np.zeros((128, 16), np.float32)], axis=1)  # [128, 64]
    c1 = b5 @ Wa0                                          # [16]
    A_ps = Wr1[0:32] + Wr1[32:64] / P
    A_pa = Wr1[64:96]
    A_sp = Wr1[96:112] + Wr1[112:128] / N
    asc = np.stack([Wr1[128] / 48.0, Wr1[129] / P,
                    A_ps.T @ b5, A_pa.T @ b5]).astype(np.float32)  # [4, 128]
    br1_eff = br1 + N * (A_sp.T @ bs3)

    gsp = np.zeros((49, 16 * 64), np.float32)
    for n in range(N):
        for d in range(3):
            gsp[n * 3 + d, n * 64:(n + 1) * 64] = Ws1[d]
        gsp[48, n * 64:(n + 1) * 64] = Ws1[3] * SPIN_F[n] + bs1

    col = lambda v: np.ascontiguousarray(v.reshape(-1, 1), dtype=np.float32)
    shared = dict(
        gsel=_gsel_const(),
        w1=W1, w2=W2, w3=W3, w4=W4, w5p=W5p,
        b1=col(b1), b2=col(b2), b3=col(b3), b4=col(b4),
        wa1bc=np.tile(Wa1.reshape(1, 16), (P, 1)).astype(np.float32),
        c1ba0=np.tile(np.concatenate([c1, ba0]).reshape(1, 32), (P, 1)).astype(np.float32),
        pmc=np.stack([SPIN_MATCH, LNGAMMA], axis=1).astype(np.float32),
        gsp=gsp, ws2=Ws2, bs2=col(bs2), ws3=Ws3,
        aps=A_ps, apa=A_pa, asp=A_sp, asc=asc,
        br1=col(br1_eff), wr2=Wr2, br2=col(br2),
        wr3=Wr3, br3=col(br3), wr4=Wr4.reshape(128, 1), br4=br4.reshape(1, 1),
    )
    shared = {k: np.ascontiguousarray(v, dtype=np.float32)
              for k, v in shared.items()}
    in_maps = []
    for core in range(NCORES):
        xc = x[core * BC:(core + 1) * BC]                     # [BC, 16, 3]
        xT = np.concatenate([xc.transpose(1, 2, 0).reshape(48, BC),
                             np.ones((1, BC), np.float32)], axis=0)
        in_maps.append({**shared, "xT": np.ascontiguousarray(xT, np.float32)})
    return in_maps


def run(in_maps, trace=False, trace_kwargs=None):
    if trace:
        _install_ntff_hook()
    return run_bass_kernel_spmd(_get_nc(), in_maps, list(range(NCORES)),
                                trace=trace, **(trace_kwargs or {}))


def _install_ntff_hook():
    if "antenv.axon_hooks" in sys.modules:
        return
    m = types.ModuleType("antenv.axon_hooks")
    m._hook = None
    m.set_axon_ntff_profile_hook = lambda h: setattr(m, "_hook", h)
    m.get_axon_ntff_profile_hook = lambda: m._hook
    sys.modules["antenv.axon_hooks"] = m
    import antenv
    antenv.axon_hooks = m
    from trn_agent_boot.trn_boot import _ntff_profile_via_ctypes
    m.set_axon_ntff_profile_hook(_ntff_profile_via_ctypes("/opt/axon/libaxon_pjrt.so"))


def kernel(x, pair_params, attn_params, sp_params, readout_params):
    in_maps = host_inputs(x, pair_params, attn_params, sp_params, readout_params)
    res = run(in_maps)
    out = np.concatenate([res.results[i]["y"] for i in range(NCORES)], axis=0)
    return np.ascontiguousarray(out, dtype=np.float32)


# revision 27
# speedup vs baseline: 1.3612x; 1.1468x over previous
"""DeepSet Jastrow factor kernel for Trainium2 (8 NeuronCores, data parallel).

Self-contained: builds a Bass/Tile kernel once (cached), shards the batch
across 8 cores, runs via run_bass_kernel_spmd, gathers the output.
"""

import sys
import types
from contextlib import ExitStack

sys.path.insert(0, "/opt/trn_rl_repo")

import numpy as np

import concourse.bass as bass
import concourse.tile as tile
from concourse import bacc, mybir
from concourse.bass_utils import run_bass_kernel_spmd

F32 = mybir.dt.float32
F32R = mybir.dt.float32r
BF16 = mybir.dt.bfloat16
AF = mybir.ActivationFunctionType
OP = mybir.AluOpType
AX = mybir.AxisListType

# ---------------- problem constants (hardcoded per spec) ----------------
B, N, D = 4096, 16, 3
P = N * (N - 1) // 2  # 120
NCORES = 8
BC = B // NCORES  # 512 samples per core
W = 128           # batch block width
NBLK = BC // W    # 4 blocks per core
CH = 24           # pairs per MLP chunk
NCHUNK = P // CH  # 5
FDC = CH * W      # 3072 tokens per chunk

F32EPS = float(np.finfo(np.float32).eps)
EPS2 = np.float64(0.04)        # EPS_FEAT^2, EPS_FEAT = 0.2
G2 = np.float64(0.09)          # GATE_R^2, GATE_R = 0.3

A_S1 = float(1.0 / EPS2)                      # 25
C_S1 = float((F32EPS + 2 * EPS2) / EPS2)      # ln(1+rt2) arg
C_W = float((F32EPS + EPS2) / EPS2)           # ln(rt2) arg
A_G = float(1.0 / G2)
C_G = float((F32EPS + G2) / G2)

IDX_I, IDX_J = np.triu_indices(N, k=1)
_spin = np.concatenate([np.zeros(N // 2), np.ones(N - N // 2)])
SPIN_MATCH = (_spin[IDX_I] == _spin[IDX_J]).astype(np.float32)
GAMMA = SPIN_MATCH * (1.0 / (D + 1)) + (1.0 - SPIN_MATCH) * (1.0 / (D - 1))
LNGAMMA = np.log(GAMMA).astype(np.float32)
SPIN_F = _spin.astype(np.float32)


def _gsel_const():
    """[48, 720] six stacked selection matrices: diff d0..2, cent d0..2."""
    G = np.zeros((6, 48, P), np.float32)
    for p in range(P):
        i, j = int(IDX_I[p]), int(IDX_J[p])
        for d in range(3):
            G[d, i * 3 + d, p] += 1.0
            G[d, j * 3 + d, p] -= 1.0
            G[3 + d, i * 3 + d, p] += 0.5
            G[3 + d, j * 3 + d, p] += 0.5
    return np.concatenate(list(G), axis=1)  # [48, 720]


# ---------------- device program ----------------

def _emit(tc, ctx, io):
    nc = tc.nc
    cst = ctx.enter_context(tc.tile_pool(name="cst", bufs=1))
    pm = ctx.enter_context(tc.tile_pool(name="pm", bufs=2))
    pm1 = ctx.enter_context(tc.tile_pool(name="pm1", bufs=1))
    pinp = ctx.enter_context(tc.tile_pool(name="pinp", bufs=2))
    hp = ctx.enter_context(tc.tile_pool(name="hp", bufs=2))
    drp = ctx.enter_context(tc.tile_pool(name="drp", bufs=2, space="DRAM"))
    mmps = ctx.enter_context(tc.tile_pool(name="mmps", bufs=2, space="PSUM"))
    smps = ctx.enter_context(tc.tile_pool(name="smps", bufs=2, space="PSUM"))

    def mm_ps(shape):
        return mmps.tile(shape, F32, tag="mm", name="mmtile")

    def sm_ps(shape):
        return smps.tile(shape, F32, tag="sm", name="smtile")

    # ---- load constants/weights to SBUF ----
    def load(name, shape, dtype=F32):
        t = cst.tile(shape, dtype, name=name)
        nc.sync.dma_start(out=t[:], in_=io[name][:])
        return t

    xTs = load("xT", [49, BC])
    gselS = load("gsel", [48, 720])
    w1f = load("w1", [13, 128])
    w2f = load("w2", [128, 128])
    w3f = load("w3", [128, 128])
    w4f = load("w4", [128, 128])
    w5f = load("w5p", [128, 64])
    b1S = load("b1", [128, 1])
    b2S = load("b2", [128, 1])
    b3S = load("b3", [128, 1])
    b4S = load("b4", [128, 1])
    wa1S = load("wa1bc", [120, 16])
    cbS = load("c1ba0", [120, 32])
    pmcS = load("pmc", [120, 2])
    gspS = load("gsp", [49, 1024])
    ws2S = load("ws2", [64, 64])
    bs2S = load("bs2", [64, 1])
    ws3S = load("ws3", [64, 16])
    apsS = load("aps", [32, 128])
    apaS = load("apa", [32, 128])
    aspS = load("asp", [16, 128])
    ascS = load("asc", [4, 128])
    br1S = load("br1", [128, 1])
    wr2S = load("wr2", [128, 128])
    br2S = load("br2", [128, 1])
    wr3S = load("wr3", [128, 128])
    br3S = load("br3", [128, 1])
    wr4S = load("wr4", [128, 1])
    br4S = load("br4", [1, 1])

    # fp32r-rounded pair-MLP weights
    def round_r(src, shape, name):
        t = cst.tile(shape, BF16, name=name)
        nc.vector.tensor_copy(out=t[:], in_=src[:])
        return t

    w1s = round_r(w1f, [13, 128], "w1s")
    w2s = round_r(w2f, [128, 128], "w2s")
    w3s = round_r(w3f, [128, 128], "w3s")
    w4s = round_r(w4f, [128, 128], "w4s")
    w5s = round_r(w5f, [128, 64], "w5s")

    ones120 = cst.tile([120, 1], F32)
    nc.vector.memset(ones120[:], 1.0)
    ones48 = cst.tile([48, 1], F32)
    nc.vector.memset(ones48[:], 1.0)
    onesr = cst.tile([120, 1], BF16)
    nc.vector.tensor_copy(out=onesr[:], in_=ones120[:])

    def cbias(val, name):
        t = cst.tile([120, 1], F32, name=name)
        nc.vector.memset(t[:], val)
        return t

    ones_row = cst.tile([1, 32], F32)
    nc.vector.memset(ones_row[:], 1.0)
    cb16 = cst.tile([120, 32], BF16)
    nc.vector.tensor_copy(out=cb16[:], in_=cbS[:])
    wa116 = cst.tile([120, 16], BF16)
    nc.vector.tensor_copy(out=wa116[:], in_=wa1S[:])

    xb16 = cst.tile([49, BC], BF16)
    nc.vector.tensor_copy(out=xb16[:], in_=xTs[:])
    gsp16 = round_r(gspS, [49, 1024], "gsp16")
    ws2b = round_r(ws2S, [64, 64], "ws2b")
    ws3b = round_r(ws3S, [64, 16], "ws3b")
    aps16 = round_r(apsS, [32, 128], "aps16")
    apa16 = round_r(apaS, [32, 128], "apa16")
    asp16 = round_r(aspS, [16, 128], "asp16")
    asc16 = round_r(ascS, [4, 128], "asc16")
    wr2b = round_r(wr2S, [128, 128], "wr2b")
    wr3b = round_r(wr3S, [128, 128], "wr3b")
    wr4b = round_r(wr4S, [128, 1], "wr4b")

    cb_s1 = cbias(C_S1, "cb_s1")
    cb_w = cbias(C_W, "cb_w")
    cb_wn = cbias(-C_W, "cb_wn")
    cb_g = cbias(C_G, "cb_g")
    cb_tiny = cbias(1e-30, "cb_tiny")

    y_d = io["y"]

    st = [dict() for _ in range(NBLK)]

    def stage_a(blk):
        bo = blk * W
        s = st[blk]
        # pair geometry + scalar features
        dc = pm.tile([120, 6, W], F32, tag="dc", name="dc")
        feat = pm.tile([120, 13, W], BF16, tag="feat", name="feat", bufs=4)
        scp = pm.tile([120, 3, W], F32, tag="scp", name="scp", bufs=4)
        s["feat"], s["scp"] = feat, scp
        for i in range(6):
            ps = mm_ps([120, W])
            nc.tensor.matmul(ps[:], gselS[:, i * 120:(i + 1) * 120],
                             xTs[0:48, bo:bo + W], start=True, stop=True)
            nc.vector.tensor_copy(out=dc[:, i, :], in_=ps[:])
            nc.vector.tensor_copy(out=feat[:, 6 + i, :], in_=ps[:])

        r2 = pm.tile([120, W], F32, tag="r2", name="r2")
        sq = pm.tile([120, W], F32, tag="sq", name="sq")
        nc.vector.tensor_tensor(out=r2[:], in0=dc[:, 0, :], in1=dc[:, 0, :], op=OP.mult)
        nc.vector.tensor_tensor(out=sq[:], in0=dc[:, 1, :], in1=dc[:, 1, :], op=OP.mult)
        nc.vector.tensor_tensor(out=r2[:], in0=r2[:], in1=sq[:], op=OP.add)
        nc.vector.tensor_tensor(out=sq[:], in0=dc[:, 2, :], in1=dc[:, 2, :], op=OP.mult)
        nc.vector.tensor_tensor(out=r2[:], in0=r2[:], in1=sq[:], op=OP.add)

        def tmp(nm):
            return pm.tile([120, W], F32, tag=nm, name=nm, bufs=4)

        # ln era: all Ln activations for this block
        nc.scalar.activation(out=scp[:, 1, :], in_=r2[:], func=AF.Ln,
                             scale=A_S1, bias=cb_s1[:])
        nc.vector.tensor_copy(out=feat[:, 0, :], in_=scp[:, 1, :])
        t1 = tmp("t1")
        nc.scalar.activation(out=t1[:], in_=r2[:], func=AF.Ln, scale=A_S1, bias=cb_w[:])
        t5 = tmp("t5")
        nc.scalar.activation(out=t5[:], in_=r2[:], func=AF.Ln, scale=A_G, bias=cb_g[:])
        t7 = tmp("t7")
        nc.scalar.activation(out=t7[:], in_=r2[:], func=AF.Ln, scale=1.0, bias=cb_tiny[:])
        s["t1"], s["t5"], s["t7"], s["r2"] = t1, t5, t7, r2
        nc.vector.tensor_copy(out=feat[:, 12, :],
                              in_=pmcS[:, 0:1].to_broadcast([120, W]))

        # r2_mean numerator
        sq48 = pm.tile([48, W], F32, tag="sq48", name="sq48")
        nc.scalar.square(out=sq48[:], in_=xTs[0:48, bo:bo + W])
        ps_r2m = sm_ps([1, W])
        nc.tensor.matmul(ps_r2m[:], ones48[:], sq48[:], start=True, stop=True)
        scinr = pm.tile([1, 4 * W], F32, tag="scinr", name="scinr", bufs=4)
        s["scinr"] = scinr
        nc.vector.tensor_copy(out=scinr[:, 0:W], in_=ps_r2m[:])

    def stage_a2(blk):
        # exp era: everything downstream of the lns
        s = st[blk]
        feat, scp = s["feat"], s["scp"]
        t1, t5, t7, r2 = s["t1"], s["t5"], s["t7"], s["r2"]

        def tmp(nm):
            return pm.tile([120, W], F32, tag=nm, name=nm, bufs=4)

        t2 = tmp("t2")
        nc.scalar.activation(out=t2[:], in_=t1[:], func=AF.Exp, scale=-1.0)
        nc.vector.tensor_scalar(out=feat[:, 1, :], in0=t2[:], scalar1=-1.0,
                                scalar2=1.0, op0=OP.mult, op1=OP.add)
        t3 = tmp("t3")
        nc.scalar.activation(out=t3[:], in_=r2[:], func=AF.Exp, scale=-A_S1, bias=cb_wn[:])
        t4 = tmp("t4")
        nc.vector.tensor_scalar(out=t4[:], in0=r2[:], scalar1=A_S1, scalar2=C_W,
                                op0=OP.mult, op1=OP.add)
        nc.vector.tensor_tensor(out=feat[:, 2, :], in0=t4[:], in1=t3[:], op=OP.mult)
        for j, g in enumerate((0.25, 1.0, 4.0)):
            nc.scalar.activation(out=feat[:, 3 + j, :], in_=scp[:, 1, :],
                                 func=AF.Exp, scale=-g)
        t6 = tmp("t6")
        nc.scalar.activation(out=t6[:], in_=t5[:], func=AF.Exp, scale=-1.0)
        nc.vector.tensor_scalar(out=scp[:, 0, :], in0=t6[:], scalar1=-1.0,
                                scalar2=1.0, op0=OP.mult, op1=OP.add)
        t8 = tmp("t8")
        nc.scalar.activation(out=t8[:], in_=t7[:], func=AF.Exp, scale=0.5)
        t9 = tmp("t9")
        nc.vector.scalar_tensor_tensor(out=t9[:], in0=t7[:], scalar=0.5, in1=t8[:],
                                       op0=OP.mult, op1=OP.subtract)
        nc.scalar.activation(out=scp[:, 2, :], in_=t9[:], func=AF.Exp,
                             scale=1.0, bias=pmcS[:, 1:2])

    def stage_sp(blk):
        bo = blk * W
        s = st[blk]
        hs1 = pm1.tile([64, 16 * W], BF16, tag="hs1", name="hs1", bufs=2)
        hs2 = pm1.tile([64, 16 * W], BF16, tag="hs2", name="hs2", bufs=2)
        for half in range(2):
            ps = mm_ps([64, 1024])
            for nl in range(8):
                n = half * 8 + nl
                nc.tensor.matmul(ps[:, nl * W:(nl + 1) * W],
                                 gsp16[:, n * 64:(n + 1) * 64],
                                 xb16[:, bo:bo + W], start=True, stop=True)
            nc.scalar.activation(out=hs1[:, half * 1024:(half + 1) * 1024], in_=ps[:],
                                 func=AF.Gelu)
        for half in range(2):
            ps = mm_ps([64, 1024])
            for ss in range(2):
                nc.tensor.matmul(ps[:, ss * 512:(ss + 1) * 512], ws2b[:],
                                 hs1[:, half * 1024 + ss * 512:half * 1024 + (ss + 1) * 512],
                                 start=True, stop=True)
            nc.scalar.activation(out=hs2[:, half * 1024:(half + 1) * 1024], in_=ps[:],
                                 func=AF.Gelu, bias=bs2S[0:64, :])
        sp16 = pm.tile([16, W], BF16, tag="sp16", name="sp16", bufs=4)
        s["sp16"] = sp16
        spp = pm.tile([16, 2, W], F32, tag="spp", name="spp")
        for half in range(2):
            ps = mm_ps([16, 1024])
            for ss in range(2):
                nc.tensor.matmul(ps[:, ss * 512:(ss + 1) * 512], ws3b[:],
                                 hs2[:, half * 1024 + ss * 512:half * 1024 + (ss + 1) * 512],
                                 start=True, stop=True)
            nc.vector.tensor_reduce(out=spp[:, half, :],
                                    in_=ps[:].rearrange("f (n b) -> f b n", n=8),
                                    axis=AX.X, op=OP.add)
        nc.vector.tensor_tensor(out=sp16[:], in0=spp[:, 0, :], in1=spp[:, 1, :], op=OP.add)

    def stage_b(blk):
        s = st[blk]
        feat = s["feat"]
        raw = pm1.tile([120, 48, W], BF16, tag="raw", name="raw", bufs=2)
        s["raw"] = raw
        for c in range(NCHUNK):
            p0 = c * CH
            pin = pinp.tile([13, FDC], BF16, tag="pin", name="pin")
            dfeat = drp.tile([CH, 13, W], BF16, tag="dfeat", name="dfeat")
            nc.sync.dma_start(out=dfeat[:], in_=feat[p0:p0 + CH, :, :])
            nc.sync.dma_start(
                out=pin[:].rearrange("c (p b) -> c p b", p=CH),
                in_=dfeat[:].rearrange("p c b -> c p b"))

            hcur = pin
            for wS, bS in ((w1s, b1S), (w2s, b2S), (w3s, b3S), (w4s, b4S)):
                hnext = hp.tile([128, FDC], BF16, tag="h", name="h")
                for ss2 in range(2):
                    ps = mm_ps([128, 1536])
                    for ss in range(3):
                        off = ss2 * 1536 + ss * 512
                        nc.tensor.matmul(ps[:, ss * 512:(ss + 1) * 512], wS[:],
                                         hcur[:, off:off + 512], start=True, stop=True)
                    nc.scalar.activation(out=hnext[:, ss2 * 1536:(ss2 + 1) * 1536],
                                         in_=ps[:], func=AF.Gelu, bias=bS[:])
                hcur = hnext
            stg = pinp.tile([128, 1536], BF16, tag="stg", name="stg")
            for ss2 in range(2):
                po = 64 * ss2
                ps5 = mm_ps([64, 1536])
                for ss in range(3):
                    off = ss2 * 1536 + ss * 512
                    nc.tensor.matmul(ps5[:, ss * 512:(ss + 1) * 512], w5s[:],
                                     hcur[:, off:off + 512], start=True, stop=True)
                for ss in range(3):
                    nc.vector.tensor_copy(
                        out=stg[po:po + 64, ss * 512:(ss + 1) * 512],
                        in_=ps5[:, ss * 512:(ss + 1) * 512])
            for ss2 in range(2):
                po = 64 * ss2
                draw = drp.tile([48, 12, W], BF16, tag="draw", name="draw")
                nc.sync.dma_start(out=draw[:], in_=stg[po:po + 48, :])
                nc.sync.dma_start(
                    out=raw[p0 + 12 * ss2:p0 + 12 * ss2 + 12, :, :],
                    in_=draw[:].rearrange("f p b -> p f b"))

    def stage_c(blk):
        s = st[blk]
        raw, scp, scinr = s["raw"], s["scp"], s["scinr"]
        gate_b = pm.tile([120, W], BF16, tag="gate_b", name="gate_b")
        nc.vector.tensor_copy(out=gate_b[:], in_=scp[:, 0, :])
        gate1 = gate_b[:].rearrange("p (f b) -> p f b", f=1)
        emb = pm1.tile([120, 32, W], BF16, tag="emb", name="emb")
        nc.vector.tensor_tensor(out=emb[:], in0=raw[:, 0:32, :],
                                in1=gate1.to_broadcast([120, 32, W]), op=OP.mult)
        ps32 = pm.tile([32, W], BF16, tag="ps32", name="ps32")
        s["ps32"] = ps32
        embf = emb[:].rearrange("p f b -> p (f b)")
        for ss in range(8):
            ps = sm_ps([1, 512])
            nc.tensor.matmul(ps[:], onesr[:], embf[:, ss * 512:(ss + 1) * 512],
                             start=True, stop=True)
            prow = pm.tile([1, 512], BF16, tag="prow", name="prow", bufs=3)
            nc.vector.tensor_copy(out=prow[:], in_=ps[:])
            nc.sync.dma_start(out=ps32[4 * ss:4 * ss + 4, :], in_=prow[:])

        u16g = pm1.tile([120, 16, W], BF16, tag="u16g", name="u16g")
        c1b = cb16[:, 0:16].rearrange("p (f b) -> p f b", b=1).to_broadcast([120, 16, W])
        ba0b = cb16[:, 16:32].rearrange("p (f b) -> p f b", b=1).to_broadcast([120, 16, W])
        nc.vector.tensor_tensor(out=u16g[:], in0=raw[:, 32:48, :], in1=c1b, op=OP.add)
        nc.vector.tensor_tensor(out=u16g[:], in0=u16g[:],
                                in1=gate1.to_broadcast([120, 16, W]), op=OP.mult)
        nc.vector.tensor_tensor(out=u16g[:], in0=u16g[:], in1=ba0b, op=OP.add)
        nc.scalar.activation(out=u16g[:], in_=u16g[:], func=AF.Tanh)
        wab = wa116[:].rearrange("p (f b) -> p f b", b=1).to_broadcast([120, 16, W])
        nc.vector.tensor_tensor(out=u16g[:], in0=u16g[:], in1=wab, op=OP.mult)
        acc = pm.tile([120, W], F32, tag="acc", name="acc")
        nc.vector.tensor_reduce(
            out=acc[:], in_=u16g[:].rearrange("p f b -> p b f"),
            axis=AX.X, op=OP.add)
        sc2 = pm.tile([120, 2, W], F32, tag="sc2", name="sc2")
        nc.scalar.activation(out=sc2[:, 1, :], in_=acc[:], func=AF.Exp)
        nc.vector.tensor_tensor(out=sc2[:, 0, :], in0=scp[:, 0, :], in1=sc2[:, 1, :],
                                op=OP.mult)
        gexp_b = pm.tile([120, W], BF16, tag="gexp_b", name="gexp_b")
        nc.vector.tensor_copy(out=gexp_b[:], in_=sc2[:, 0, :])
        gexp1 = gexp_b[:].rearrange("p (f b) -> p f b", f=1)
        nc.vector.tensor_tensor(out=emb[:], in0=raw[:, 0:32, :],
                                in1=gexp1.to_broadcast([120, 32, W]), op=OP.mult)
        pe32 = pm.tile([32, W], BF16, tag="pe32", name="pe32")
        embxf = emb[:].rearrange("p f b -> p (f b)")
        for ss in range(8):
            ps = sm_ps([1, 512])
            nc.tensor.matmul(ps[:], onesr[:], embxf[:, ss * 512:(ss + 1) * 512],
                             start=True, stop=True)
            prow2 = pm.tile([1, 512], BF16, tag="prow", name="prow2", bufs=3)
            nc.vector.tensor_copy(out=prow2[:], in_=ps[:])
            nc.sync.dma_start(out=pe32[4 * ss:4 * ss + 4, :], in_=prow2[:])

        ps_sc = sm_ps([1, 3 * W])
        nc.tensor.matmul(ps_sc[:], ones120[:],
                         scp[:].rearrange("p c b -> p (c b)"), start=True, stop=True)
        cuspr = pm.tile([1, W], F32, tag="cuspr", name="cuspr", bufs=4)
        s["cuspr"] = cuspr
        nc.vector.tensor_copy(out=scinr[:, 2 * W:3 * W], in_=ps_sc[:, 0:W])
        nc.vector.tensor_copy(out=scinr[:, W:2 * W], in_=ps_sc[:, W:2 * W])
        nc.vector.tensor_copy(out=cuspr[:], in_=ps_sc[:, 2 * W:3 * W])
        ps_sc2 = sm_ps([1, 2 * W])
        nc.tensor.matmul(ps_sc2[:], ones120[:],
                         sc2[:].rearrange("p c b -> p (c b)"), start=True, stop=True)
        rd = pm.tile([1, W], F32, tag="rd", name="rd")
        nc.vector.reciprocal(out=rd[:], in_=ps_sc2[:, W:2 * W])
        nc.vector.tensor_tensor(out=scinr[:, 3 * W:4 * W], in0=ps_sc2[:, 0:W],
                                in1=rd[:], op=OP.mult)
        scinb = pm.tile([1, 4 * W], BF16, tag="scinb", name="scinb")
        nc.vector.tensor_copy(out=scinb[:], in_=scinr[:])
        scin = pm.tile([4, W], BF16, tag="scin", name="scin")
        s["scin"] = scin
        nc.sync.dma_start(out=scin[:], in_=scinb[:])
        ps_rb = sm_ps([32, W])
        nc.tensor.matmul(ps_rb[:], ones_row[:], rd[:], start=True, stop=True)
        attn32 = pm.tile([32, W], BF16, tag="attn32", name="attn32")
        s["attn32"] = attn32
        nc.vector.tensor_tensor(out=attn32[:], in0=pe32[:], in1=ps_rb[:], op=OP.mult)

    def stage_d(blk):
        bo = blk * W
        s = st[blk]
        psR = mm_ps([128, W])
        nc.tensor.matmul(psR[:], aps16[:], s["ps32"][:], start=True, stop=False)
        nc.tensor.matmul(psR[:], apa16[:], s["attn32"][:], start=False, stop=False)
        nc.tensor.matmul(psR[:], asp16[:], s["sp16"][:], start=False, stop=False)
        nc.tensor.matmul(psR[:], asc16[:], s["scin"][:], start=False, stop=True)
        h1R = pm.tile([128, W], BF16, tag="h1R", name="h1R")
        nc.scalar.activation(out=h1R[:], in_=psR[:], func=AF.Gelu, bias=br1S[:])
        psR2 = mm_ps([128, W])
        nc.tensor.matmul(psR2[:], wr2b[:], h1R[:], start=True, stop=True)
        h2R = pm.tile([128, W], BF16, tag="h2R", name="h2R")
        nc.scalar.activation(out=h2R[:], in_=psR2[:], func=AF.Gelu, bias=br2S[:])
        psR3 = mm_ps([128, W])
        nc.tensor.matmul(psR3[:], wr3b[:], h2R[:], start=True, stop=True)
        h3R = pm.tile([128, W], BF16, tag="h3R", name="h3R")
        nc.scalar.activation(out=h3R[:], in_=psR3[:], func=AF.Gelu, bias=br3S[:])
        psL4 = sm_ps([1, W])
        nc.tensor.matmul(psL4[:], wr4b[:], h3R[:], start=True, stop=True)
        ys = pm.tile([1, W], F32, tag="ys", name="ys")
        nc.vector.scalar_tensor_tensor(out=ys[:], in0=psL4[:], scalar=br4S[0:1, 0:1],
                                       in1=s["cuspr"][:], op0=OP.add, op1=OP.add)
        nc.sync.dma_start(out=y_d[bo:bo + W, 0:1], in_=ys[:])

    # features for every block first (one ln/exp table era), then the
    # gelu-heavy MLP stages pipelined with attention/readout
    for b in range(NBLK):
        stage_a(b)
    for b in range(NBLK):
        stage_a2(b)
    stage_b(0)
    stage_sp(0)
    stage_b(1)
    stage_sp(1)
    stage_c(0)
    stage_b(2)
    stage_sp(2)
    stage_d(0)
    stage_c(1)
    stage_b(3)
    stage_sp(3)
    stage_d(1)
    stage_c(2)
    stage_c(3)
    stage_d(2)
    stage_d(3)


def build():
    nc = bacc.Bacc("TRN2", target_bir_lowering=False, debug=False,
                   num_devices=NCORES)
    io = {}

    def din(name, shape, dtype=F32):
        io[name] = nc.dram_tensor(name, shape, dtype, kind="ExternalInput").ap()

    din("xT", [49, BC])
    din("gsel", [48, 720])
    din("w1", [13, 128])
    din("w2", [128, 128])
    din("w3", [128, 128])
    din("w4", [128, 128])
    din("w5p", [128, 64])
    for nm in ("b1", "b2", "b3", "b4"):
        din(nm, [128, 1])
    din("wa1bc", [120, 16])
    din("c1ba0", [120, 32])
    din("pmc", [120, 2])
    din("gsp", [49, 1024])
    din("ws2", [64, 64])
    din("bs2", [64, 1])
    din("ws3", [64, 16])
    din("aps", [32, 128])
    din("apa", [32, 128])
    din("asp", [16, 128])
    din("asc", [4, 128])
    din("br1", [128, 1])
    din("wr2", [128, 128])
    din("br2", [128, 1])
    din("wr3", [128, 128])
    din("br3", [128, 1])
    din("wr4", [128, 1])
    din("br4", [1, 1])
    io["y"] = nc.dram_tensor("y", [BC, 1], F32, kind="ExternalOutput").ap()

    with tile.TileContext(nc) as tc, ExitStack() as ctx:
        _emit(tc, ctx, io)
    nc.compile()
    return nc


_NC = None


def _get_nc():
    global _NC
    if _NC is None:
        _NC = build()
    return _NC


def host_inputs(x, pair_params, attn_params, sp_params, readout_params):
    """Build shared weight map + per-core input maps."""
    a = lambda v: np.asarray(v, np.float32)
    x = a(x)
    (W1, b1), (W2, b2), (W3, b3), (W4, b4), (W5, b5) = \
        [(a(w), a(b)) for w, b in pair_params]
    (Wa0, ba0), (Wa1, ba1) = [(a(w), a(b)) for w, b in attn_params]
    (Ws1, bs1), (Ws2, bs2), (Ws3, bs3) = [(a(w), a(b)) for w, b in sp_params]
    (Wr1, br1), (Wr2, br2), (Wr3, br3), (Wr4, br4) = \
        [(a(w), a(b)) for w, b in readout_params]

    W5p = np.concatenate([W5, W5 @ Wa0,
                          np.zeros((128, 16), np.float32)], axis=1)  # [128, 64]
    c1 = b5 @ Wa0                                          # [16]
    A_ps = Wr1[0:32] + Wr1[32:64] / P
    A_pa = Wr1[64:96]
    A_sp = Wr1[96:112] + Wr1[112:128] / N
    asc = np.stack([Wr1[128] / 48.0, Wr1[129] / P,
                    A_ps.T @ b5, A_pa.T @ b5]).astype(np.float32)  # [4, 128]
    br1_eff = br1 + N * (A_sp.T @ bs3)

    gsp = np.zeros((49, 16 * 64), np.float32)
    for n in range(N):
        for d in range(3):
            gsp[n * 3 + d, n * 64:(n + 1) * 64] = Ws1[d]
        gsp[48, n * 64:(n + 1) * 64] = Ws1[3] * SPIN_F[n] + bs1

    col = lambda v: np.ascontiguousarray(v.reshape(-1, 1), dtype=np.float32)
    shared = dict(
        gsel=_gsel_const(),
        w1=W1, w2=W2, w3=W3, w4=W4, w5p=W5p,
        b1=col(b1), b2=col(b2), b3=col(b3), b4=col(b4),
        wa1bc=np.tile(Wa1.reshape(1, 16), (P, 1)).astype(np.float32),
        c1ba0=np.tile(np.concatenate([c1, ba0]).reshape(1, 32), (P, 1)).astype(np.float32),
        pmc=np.stack([SPIN_MATCH, LNGAMMA], axis=1).astype(np.float32),
        gsp=gsp, ws2=Ws2, bs2=col(bs2), ws3=Ws3,
        aps=A_ps, apa=A_pa, asp=A_sp, asc=asc,
        br1=col(br1_eff), wr2=Wr2, br2=col(br2),
        wr3=Wr3, br3=col(br3), wr4=Wr4.reshape(128, 1), br4=br4.reshape(1, 1),
    )
    shared = {k: np.ascontiguousarray(v, dtype=np.float32)
              for k, v in shared.items()}
    in_maps = []
    for core in range(NCORES):
        xc = x[core * BC:(core + 1) * BC]                     # [BC, 16, 3]
        xT = np.concatenate([xc.transpose(1, 2, 0).reshape(48, BC),
                             np.ones((1, BC), np.float32)], axis=0)
        in_maps.append({**shared, "xT": np.ascontiguousarray(xT, np.float32)})
    return in_maps


def run(in_maps, trace=False, trace_kwargs=None):
    if trace:
        _install_ntff_hook()
    return run_bass_kernel_spmd(_get_nc(), in_maps, list(range(NCORES)),
                                trace=trace, **(trace_kwargs or {}))


def _install_ntff_hook():
    if "antenv.axon_hooks" in sys.modules:
        return
    m = types.ModuleType("antenv.axon_hooks")
    m._hook = None
    m.set_axon_ntff_profile_hook = lambda h: setattr(m, "_hook", h)
    m.get_axon_ntff_profile_hook = lambda: m._hook
    sys.modules["antenv.axon_hooks"] = m
    import antenv
    antenv.axon_hooks = m
    from trn_agent_boot.trn_boot import _ntff_profile_via_ctypes
    m.set_axon_ntff_profile_hook(_ntff_profile_via_ctypes("/opt/axon/libaxon_pjrt.so"))


def kernel(x, pair_params, attn_params, sp_params, readout_params):
    in_maps = host_inputs(x, pair_params, attn_params, sp_params, readout_params)
    res = run(in_maps)
    out = np.concatenate([res.results[i]["y"] for i in range(NCORES)], axis=0)
    return np.ascontiguousarray(out, dtype=np.float32)
